# revision 1
# baseline (speedup 1.0000x reference)
"""Trainium2 Bass kernel for nn_BlockDirectTwice (dual-branch transformer block).

Sharding: data-parallel over batch. 8 batch elements -> 8 NeuronCores; every
core runs the full block (two LN+MHA branches, blend, LN, MLP, residuals) on
its own [S, D] slab. No collectives.

Numerics: matmuls in bf16 (fp32 PSUM accumulation); LayerNorm statistics,
softmax denominators and residual stream kept in fp32.
"""

import numpy as np
import ml_dtypes

B, S, D, H, DFF = 8, 1024, 768, 12, 3072
HD = D // H          # 64
P = 128
KD = D // P          # 6  K-subtiles over D
KF = DFF // P        # 24 K-subtiles over DFF
NT = S // P          # 8  token tiles
NPAIR = H // 2       # 6  head pairs
UP, MID = 0.6, 0.4
EPS = 1e-6
N_CORES = 8
ATT_SCALE = 1.0 / np.sqrt(HD)  # 0.125

_CACHE = {}


def _split_cols(n):
    """Split n output columns into <=512 chunks."""
    out, c = [], 0
    while c < n:
        w = min(512, n - c)
        out.append((c, w))
        c += w
    return out


def _build_nc(cfg):
    """Build the per-core Bass program. cfg is a frozenset of feature flags."""
    from contextlib import ExitStack

    import concourse.bass as bass
    import concourse.tile as tile
    from concourse import bacc, mybir

    F32 = mybir.dt.float32
    BF16 = mybir.dt.bfloat16
    AF = mybir.ActivationFunctionType
    ALU = mybir.AluOpType

    has = lambda f: f in cfg
    repeat = 1
    for f in cfg:
        if f.startswith("repeat="):
            repeat = int(f.split("=")[1])

    nc = bacc.Bacc("TRN2", target_bir_lowering=False, debug=False)

    # ---------------- DRAM I/O ----------------
    x_dram = [
        nc.dram_tensor("x0", (S, D), F32, kind="ExternalInput"),
        nc.dram_tensor("x1", (S, D), F32, kind="ExternalInput"),
    ]
    w_dram = {}
    for br in (0, 1):
        for nm in ("wq", "wk", "wv", "wo"):
            w_dram[(br, nm)] = nc.dram_tensor(f"a{br}_{nm}", (D, D), BF16,
                                              kind="ExternalInput")
    fc1_dram = nc.dram_tensor("fc1_w", (D, DFF), BF16, kind="ExternalInput")
    fc2_dram = nc.dram_tensor("fc2_w", (DFF, D), BF16, kind="ExternalInput")

    # optional non-trivial params (most are zeros/ones in this problem)
    opt_dram = {}
    for name, shape in [
        ("ln0_g", (D,)), ("ln0_b", (D,)), ("ln1_g", (D,)), ("ln1_b", (D,)),
        ("lnf_g", (D,)), ("lnf_b", (D,)),
        ("fc1_b", (DFF,)), ("fc2_b", (D,)),
        ("a0_bq", (D,)), ("a0_bk", (D,)), ("a0_bv", (D,)), ("a0_bo", (D,)),
        ("a1_bq", (D,)), ("a1_bk", (D,)), ("a1_bv", (D,)), ("a1_bo", (D,)),
    ]:
        if has(name):
            opt_dram[name] = nc.dram_tensor(name, shape, F32, kind="ExternalInput")

    out_dram = nc.dram_tensor("out", (S, D), F32, kind="ExternalOutput")

    def bcast_rows(src_ap, nparts):
        """DRAM row [1, n] (or [n]) -> AP broadcast over nparts partitions."""
        ap = list(src_ap.ap)
        if len(src_ap.shape) == 1:
            ap = [[0, nparts]] + ap
        else:
            ap = [[0, nparts]] + ap[1:]
        return bass.AP(tensor=src_ap.tensor, offset=src_ap.offset, ap=ap)

    with ExitStack() as ctx:
        tc = ctx.enter_context(tile.TileContext(nc))

        sb = ctx.enter_context(tc.tile_pool(name="sb", bufs=1))
        wpool = ctx.enter_context(tc.tile_pool(name="w", bufs=2))
        lnp = ctx.enter_context(tc.tile_pool(name="ln", bufs=2))
        qkp = ctx.enter_context(tc.tile_pool(name="qk", bufs=3))
        xtp = ctx.enter_context(tc.tile_pool(name="xt", bufs=1))
        prp = ctx.enter_context(tc.tile_pool(name="probs", bufs=4))
        outp = ctx.enter_context(tc.tile_pool(name="out", bufs=2))
        psmm = ctx.enter_context(tc.tile_pool(name="psmm", bufs=4, space="PSUM"))
        pssc = ctx.enter_context(tc.tile_pool(name="pssc", bufs=2, space="PSUM"))
        dram = ctx.enter_context(tc.tile_pool(name="dram", bufs=1, space="DRAM"))

        loop_cm = tc.For_i(0, repeat, 1) if repeat > 1 else None
        if loop_cm is not None:
            ctx.enter_context(loop_cm)

        eps_t = sb.tile([P, 1], F32, tag="eps")
        nc.vector.memset(eps_t, EPS)

        # persistent big tensors
        h_tm = sb.tile([P, NT, D], F32, tag="h_tm")
        ctx_all = sb.tile([P, KD, S], BF16, tag="ctx")
        v_aug = sb.tile([P, NT, H * 65], BF16, tag="v")
        fc1_sb = sb.tile([P, KD, DFF], BF16, tag="fc1")
        fc2_sb = sb.tile([P, KF, D], BF16, tag="fc2")
        nc.gpsimd.dma_start(fc1_sb, fc1_dram.ap().rearrange("(ko p) n -> p ko n", p=P))
        nc.gpsimd.dma_start(fc2_sb, fc2_dram.ap().rearrange("(ko p) n -> p ko n", p=P))

        # optional broadcast tiles for per-feature (free-dim) params
        bcast_sb = {}
        for name in ("ln0_g", "ln0_b", "ln1_g", "ln1_b", "lnf_g", "lnf_b",
                     "a0_bv", "a1_bv", "a0_bo", "a1_bo", "fc2_b"):
            if has(name):
                t = sb.tile([P, D], F32, tag=f"bc_{name}")
                nc.gpsimd.dma_start(t, bcast_rows(opt_dram[name].ap(), P))
                bcast_sb[name] = t
        # per-partition bias tiles (feature-major layouts)
        pp_sb = {}
        for name, kk in (("a0_bq", KD), ("a0_bk", KD), ("a1_bq", KD),
                         ("a1_bk", KD), ("fc1_b", KF)):
            if has(name):
                t = sb.tile([P, kk], F32, tag=f"pp_{name}")
                nc.sync.dma_start(t, opt_dram[name].ap().rearrange("(m p) -> p m", p=P))
                pp_sb[name] = t
        for name in ("a0_bq", "a1_bq"):
            if name in pp_sb:  # q is pre-scaled by 1/8; scale its bias too
                nc.vector.tensor_scalar_mul(pp_sb[name], pp_sb[name], float(ATT_SCALE))

        # identity for PE-mode transposes
        from concourse.masks import make_identity
        ident = sb.tile([P, P], BF16, tag="ident")
        make_identity(nc, ident)

        # transposes alternate between the two HWDGE rings
        _ring = [0]

        def dma_T(out_ap, in_ap):
            eng = nc.sync if _ring[0] % 2 == 0 else nc.scalar
            _ring[0] += 1
            if has("notranspose"):  # diagnostic: same bytes, no xbar
                eng.dma_start(out_ap, in_ap)
                return
            eng.dma_start_transpose(out_ap, in_ap)

        def emit_ln(x_f32, xT_dest, t, gname, bname, on_pe=False):
            """LayerNorm x_f32 [P, D] (in-place scratch) -> bf16, transposed into
            xT_dest[:, j, t*128:(t+1)*128]."""
            stats = lnp.tile([P, 3, 6], F32, tag="stats")
            for sg in range(3):
                nc.vector.bn_stats(stats[:, sg, :], x_f32[:, sg * 256:(sg + 1) * 256])
            mv = lnp.tile([P, 2], F32, tag="mv")
            nc.vector.bn_aggr(mv, stats)
            # rstd = 1/sqrt(var+eps), DVE-only (quake init + 2 Newton steps):
            # keeps the ACT table set untouched (exp stays resident).
            rstd = lnp.tile([P, 1], F32, tag="rstd")
            vh = lnp.tile([P, 1], F32, tag="rs_vh")
            nc.vector.tensor_scalar(vh, mv[:, 1:2], EPS, 0.5, ALU.add, ALU.mult)
            yi = lnp.tile([P, 1], mybir.dt.int32, tag="rs_yi")
            # quake seed from the bits of u = var (Newton uses h = u/2)
            nc.vector.tensor_scalar(yi, mv[:, 1:2].bitcast(mybir.dt.int32), 1, None,
                                    ALU.logical_shift_right)
            y0 = lnp.tile([P, 1], F32, tag="rs_y0")
            nc.vector.tensor_scalar(yi, yi, -1, None, ALU.bitwise_xor)
            nc.vector.tensor_scalar(y0.bitcast(mybir.dt.int32), yi, 0x5f3759e0, None,
                                    ALU.add)
            t1 = lnp.tile([P, 1], F32, tag="rs_t1")
            for _ in range(2):
                nc.vector.tensor_tensor(t1, y0, y0, ALU.mult)
                nc.vector.tensor_tensor(t1, t1, vh, ALU.mult)
                nc.vector.tensor_scalar(t1, t1, -1.0, 1.5, ALU.mult, ALU.add)
                nc.vector.tensor_tensor(y0, y0, t1, ALU.mult)
            nc.vector.tensor_copy(rstd, y0)
            nc.vector.tensor_scalar(x_f32, x_f32, mv[:, 0:1], None, ALU.subtract)
            xln = lnp.tile([P, D], BF16, tag="xln")
            if has(gname):
                nc.vector.tensor_scalar_mul(x_f32, x_f32, rstd[:])
                nc.vector.tensor_tensor(x_f32, x_f32, bcast_sb[gname], ALU.mult)
                if has(bname):
                    nc.vector.tensor_tensor(xln, x_f32, bcast_sb[bname], ALU.add)
                else:
                    nc.vector.tensor_copy(xln, x_f32)
            elif has(bname):
                nc.vector.tensor_scalar_mul(x_f32, x_f32, rstd[:])
                nc.vector.tensor_tensor(xln, x_f32, bcast_sb[bname], ALU.add)
            else:
                nc.vector.tensor_scalar_mul(xln, x_f32, rstd[:])
            if on_pe:
                for j in range(KD):
                    pst = psmm.tile([P, 512], F32, tag="mm")
                    nc.tensor.transpose(pst[:, :P].bitcast(BF16)[:, :P], xln[:, j * P:(j + 1) * P], ident)
                    nc.vector.tensor_copy(xT_dest[:, j, t * P:(t + 1) * P],
                                          pst[:, :P].bitcast(BF16)[:, :P])
            else:
                for j in range(KD):
                    dma_T(xT_dest[:, j, t * P:(t + 1) * P], xln[:, j * P:(j + 1) * P])

        def stage_A(br, xT_dest):
            """Load x_br, LN, transpose; accumulate blend into h_tm."""
            g, b = (f"ln{br}_g", f"ln{br}_b")
            for t in range(NT):
                xt = lnp.tile([P, D], F32, tag="x_tm")
                nc.sync.dma_start(xt, x_dram[br].ap()[t * P:(t + 1) * P, :])
                if br == 0:
                    nc.vector.tensor_scalar_mul(h_tm[:, t, :], xt, UP)
                else:
                    nc.vector.scalar_tensor_tensor(h_tm[:, t, :], xt, MID,
                                                   h_tm[:, t, :], ALU.mult, ALU.add)
                emit_ln(xt, xT_dest, t, g, b, on_pe=(br == 0))

        def load_w(br, nm):
            t = wpool.tile([P, KD, D], BF16, tag="w768")
            nc.gpsimd.dma_start(t, w_dram[(br, nm)].ap().rearrange("(ko p) n -> p ko n", p=P))
            return t

        def stage_BC(br, xT, pending_wo=None):
            """V/Q/K projections + attention, with next-pair projection chunks
            interleaved into the attention t-loop so the in-order PE queue
            stays dense while ACT computes exps. Returns a closure that emits
            the wo projection (deferred into the next branch's warmup)."""
            wv = load_w(br, "wv")
            wq = load_w(br, "wq")
            v_view = v_aug[:].rearrange("p t (h c) -> p t h c", c=65)
            nc.vector.memset(v_view[:, :, :, 64:65], 1.0)
            for t in range(NT):
                for c0, cw in _split_cols(D):
                    ps = psmm.tile([P, 512], F32, tag="mm")
                    for k in range(KD):
                        nc.tensor.matmul(
                            ps[:, :cw], lhsT=xT[:, k, t * P:(t + 1) * P],
                            rhs=wv[:, k, c0:c0 + cw],
                            start=(k == 0), stop=(k == KD - 1))
                    nh = cw // HD
                    h0 = c0 // HD
                    src = ps[:, :cw].rearrange("p (h c) -> p h c", c=HD)
                    dst = v_view[:, t, h0:h0 + nh, 0:HD]
                    bias_key = f"a{br}_bv"
                    if bias_key in bcast_sb:
                        bcv = bcast_sb[bias_key][:, c0:c0 + cw].rearrange(
                            "p (h c) -> p h c", c=HD)
                        nc.vector.tensor_tensor(dst, src, bcv, ALU.add)
                    else:
                        nc.vector.tensor_copy(dst, src)
            if pending_wo is not None:
                pending_wo()
            wk = load_w(br, "wk")
            denom_dram = dram.tile([H, S], BF16)
            if has("noattn"):
                nc.vector.memset(ctx_all, 0.25)

            def proj_chunks(pr, qp, kp):
                """12 closures, each one (which, col-chunk) psum of pair pr."""
                chunks = []
                for (which, wt, dest) in (("q", wq, qp), ("k", wk, kp)):
                    for c0, cw in _split_cols(S):
                        def emit(which=which, wt=wt, dest=dest, c0=c0, cw=cw,
                                 last=False, pr=pr):
                            ps = psmm.tile([P, 512], F32, tag="mm")
                            for k in range(KD):
                                nc.tensor.matmul(
                                    ps[:, :cw], lhsT=wt[:, k, pr * P:(pr + 1) * P],
                                    rhs=xT[:, k, c0:c0 + cw],
                                    start=(k == 0), stop=(k == KD - 1))
                            if which == "q":
                                nc.vector.tensor_scalar_mul(
                                    dest[:, c0:c0 + cw], ps[:, :cw], float(ATT_SCALE))
                            else:
                                nc.vector.tensor_copy(dest[:, c0:c0 + cw], ps[:, :cw])
                            bias_key = f"a{br}_b{which}"
                            if bias_key in pp_sb and c0 + cw >= S:
                                nc.vector.tensor_scalar_add(
                                    dest, dest, pp_sb[bias_key][:, pr:pr + 1])
                        chunks.append(emit)
                return chunks

            if not has("noattn"):
                # prime pair 0 (nothing to interleave with yet)
                cur_qp = qkp.tile([P, S], BF16, tag="qpair", bufs=2, name="qp0")
                cur_kp = qkp.tile([P, S], BF16, tag="kpair", bufs=2, name="kp0")
                for ch in proj_chunks(0, cur_qp, cur_kp):
                    ch()
                for pr in range(NPAIR):
                    qp, kp = cur_qp, cur_kp
                    fillers = []
                    if pr + 1 < NPAIR:
                        cur_qp = qkp.tile([P, S], BF16, tag="qpair", bufs=2,
                                          name=f"qp{pr + 1}")
                        cur_kp = qkp.tile([P, S], BF16, tag="kpair", bufs=2,
                                          name=f"kp{pr + 1}")
                        fillers = proj_chunks(pr + 1, cur_qp, cur_kp)
                    nfill = 0
                    for n in range(2):
                        n0 = n * 512
                        ps_c = [psmm.tile([P, 512], F32, tag="mm", name=f"ps_c{hh}")
                                for hh in range(2)]

                        def ctx_step(t, pq):
                            for hh in range(2):
                                h = 2 * pr + hh
                                nc.tensor.matmul(
                                    ps_c[hh][0:65, :],
                                    lhsT=v_aug[:, t, h * 65:(h + 1) * 65],
                                    rhs=pq[:, hh, :],
                                    start=(t == 0), stop=(t == NT - 1))

                        LAG = 2
                        pending = []
                        for t in range(NT):
                            ps_s = pssc.tile([P, 2, 512], F32, tag="sc")
                            for hh in range(2):
                                b0 = hh * HD
                                nc.tensor.matmul(
                                    ps_s[:, hh, :],
                                    lhsT=kp[b0:b0 + HD, t * P:(t + 1) * P],
                                    rhs=qp[b0:b0 + HD, n0:n0 + 512],
                                    start=True, stop=True)
                            pq = prp.tile([P, 2, 512], BF16, tag="probs")
                            nc.scalar.activation(pq, ps_s, AF.Exp)
                            pending.append((t, pq))
                            if len(pending) > LAG:
                                ctx_step(*pending.pop(0))
                            # keep PE dense: one projection chunk of the next
                            # pair after (roughly) every other t-step
                            want = ((n * NT + t + 1) * len(fillers)) // (2 * NT)
                            while nfill < want:
                                fillers[nfill]()
                                nfill += 1
                        for item in pending:
                            ctx_step(*item)
                        for hh in range(2):
                            h = 2 * pr + hh
                            nc.vector.tensor_copy(
                                ctx_all[hh * HD:(hh + 1) * HD, pr, n0:n0 + 512],
                                ps_c[hh][0:HD, :])
                            dstage = lnp.tile([65, 512], BF16, tag="dstage", bufs=1)
                            nc.vector.tensor_copy(dstage[64:65, :], ps_c[hh][64:65, :])
                            nc.gpsimd.dma_start(denom_dram[h:h + 1, n0:n0 + 512],
                                              dstage[64:65, :])
                    while nfill < len(fillers):
                        fillers[nfill]()
                        nfill += 1
            # denominators -> reciprocal -> broadcast
            if not has("noattn"):
                recip_sb = sb.tile([H, S], F32, tag="recip")
                nc.gpsimd.dma_start(recip_sb, denom_dram[:])
                nc.vector.reciprocal_approx_fast(recip_sb, recip_sb)
                recip_dram = dram.tile([H, S], F32)
                nc.sync.dma_start(recip_dram, recip_sb)
                for pr in range(NPAIR):
                    rb = outp.tile([P, S], F32, tag="recipB", bufs=1)
                    for hh in range(2):
                        h = 2 * pr + hh
                        nc.gpsimd.dma_start(rb[hh * HD:(hh + 1) * HD, :],
                                            bcast_rows(recip_dram[h:h + 1, :], HD))
                    nc.vector.tensor_tensor(ctx_all[:, pr, :], ctx_all[:, pr, :],
                                            rb, ALU.mult)

            def emit_wo():
                wo = load_w(br, "wo")
                scale = UP if br == 0 else MID
                _wo_proj(br, wo, scale)

            return emit_wo

        def _wo_proj(br, wo, scale):
            bo_key = f"a{br}_bo"
            for t in range(NT):
                for c0, cw in _split_cols(D):
                    ps = psmm.tile([P, 512], F32, tag="mm")
                    for k in range(KD):
                        nc.tensor.matmul(
                            ps[:, :cw], lhsT=ctx_all[:, k, t * P:(t + 1) * P],
                            rhs=wo[:, k, c0:c0 + cw],
                            start=(k == 0), stop=(k == KD - 1))
                    if bo_key in bcast_sb:
                        tmp = lnp.tile([P, D], F32, tag="wo_tmp")
                        nc.vector.tensor_tensor(tmp[:, :cw], ps[:, :cw],
                                                bcast_sb[bo_key][:, c0:c0 + cw],
                                                ALU.add)
                        nc.vector.scalar_tensor_tensor(
                            h_tm[:, t, c0:c0 + cw], tmp[:, :cw], float(scale),
                            h_tm[:, t, c0:c0 + cw], ALU.mult, ALU.add)
                    else:
                        nc.vector.scalar_tensor_tensor(
                            h_tm[:, t, c0:c0 + cw], ps[:, :cw], float(scale),
                            h_tm[:, t, c0:c0 + cw], ALU.mult, ALU.add)

        # ---------------- emit program ----------------
        xT0 = xtp.tile([P, KD, S], BF16, tag="xT")
        stage_A(0, xT0)
        wo0 = stage_BC(0, xT0)
        xT1 = xtp.tile([P, KD, S], BF16, tag="xT")
        stage_A(1, xT1)
        wo1 = stage_BC(1, xT1, pending_wo=wo0)
        wo1()

        # LNf -> hT
        hT = xtp.tile([P, KD, S], BF16, tag="xT")
        for t in range(NT):
            hc = lnp.tile([P, D], F32, tag="x_tm")
            nc.vector.tensor_copy(hc, h_tm[:, t, :])
            emit_ln(hc, hT, t, "lnf_g", "lnf_b", on_pe=True)

        # MLP: fc1+gelu then fc2+residual, in token chunks of 256
        if has("nomlp"):
            for t in range(NT):
                o_t = outp.tile([P, D], F32, tag="out_t", bufs=1)
                nc.vector.tensor_copy(o_t, h_tm[:, t, :])
                nc.gpsimd.dma_start(out_dram.ap()[t * P:(t + 1) * P, :], o_t)
        for nn in range(4 if not has("nomlp") else 0):
            c0 = nn * 256
            gT = xtp.tile([P, KF, 256], BF16, tag="gT")
            for m in range(KF):
                ps = psmm.tile([P, 512], F32, tag="mm")
                for k in range(KD):
                    nc.tensor.matmul(ps[:, :256], lhsT=fc1_sb[:, k, m * P:(m + 1) * P],
                                     rhs=hT[:, k, c0:c0 + 256],
                                     start=(k == 0), stop=(k == KD - 1))
                bias = pp_sb["fc1_b"][:, m:m + 1] if "fc1_b" in pp_sb else 0.0
                nc.scalar.activation(gT[:, m, :], ps[:, :256], AF.Gelu, bias=bias)
            for tl in range(2):
                t = 2 * nn + tl
                o_t = outp.tile([P, D], F32, tag="out_t", bufs=1)
                for oc0, ocw in _split_cols(D):
                    ps = psmm.tile([P, 512], F32, tag="mm")
                    for k in range(KF):
                        nc.tensor.matmul(
                            ps[:, :ocw], lhsT=gT[:, k, tl * P:(tl + 1) * P],
                            rhs=fc2_sb[:, k, oc0:oc0 + ocw],
                            start=(k == 0), stop=(k == KF - 1))
                    if "fc2_b" in bcast_sb:
                        nc.vector.tensor_tensor(ps[:, :ocw], ps[:, :ocw],
                                                bcast_sb["fc2_b"][:, oc0:oc0 + ocw],
                                                ALU.add)
                    nc.vector.tensor_tensor(o_t[:, oc0:oc0 + ocw], ps[:, :ocw],
                                            h_tm[:, t, oc0:oc0 + ocw], ALU.add)
                nc.gpsimd.dma_start(out_dram.ap()[t * P:(t + 1) * P, :], o_t)

    nc.compile()
    return nc


def _prep_inputs(inputs):
    """Host-side prep: detect trivial params, cast weights to bf16."""
    bf16 = ml_dtypes.bfloat16
    cfg = set()
    arrs = {}
    for name in ("x0", "x1"):
        arrs[name] = np.ascontiguousarray(np.asarray(inputs[name], dtype=np.float32))
    for br in (0, 1):
        for nm in ("wq", "wk", "wv", "wo"):
            key = f"a{br}_{nm}"
            arrs[key] = np.ascontiguousarray(
                np.asarray(inputs[key], dtype=np.float32).astype(bf16))
    arrs["fc1_w"] = np.ascontiguousarray(
        np.asarray(inputs["fc1_w"], dtype=np.float32).astype(bf16))
    arrs["fc2_w"] = np.ascontiguousarray(
        np.asarray(inputs["fc2_w"], dtype=np.float32).astype(bf16))
    for name, trivial in [
        ("ln0_g", 1.0), ("ln0_b", 0.0), ("ln1_g", 1.0), ("ln1_b", 0.0),
        ("lnf_g", 1.0), ("lnf_b", 0.0), ("fc1_b", 0.0), ("fc2_b", 0.0),
        ("a0_bq", 0.0), ("a0_bk", 0.0), ("a0_bv", 0.0), ("a0_bo", 0.0),
        ("a1_bq", 0.0), ("a1_bk", 0.0), ("a1_bv", 0.0), ("a1_bo", 0.0),
    ]:
        a = np.asarray(inputs[name], dtype=np.float32)
        if not np.all(a == trivial):
            cfg.add(name)
            arrs[name] = np.ascontiguousarray(a)
    return cfg, arrs


def kernel(**inputs):
    from concourse.bass_utils import run_bass_kernel_spmd

    cfg, arrs = _prep_inputs(inputs)
    key = frozenset(cfg)
    if key not in _CACHE:
        _CACHE[key] = _build_nc(key)
    nc = _CACHE[key]

    shared = {k: v for k, v in arrs.items() if k not in ("x0", "x1")}
    in_maps = []
    for b in range(N_CORES):
        m = dict(shared)
        m["x0"] = np.ascontiguousarray(arrs["x0"][b])
        m["x1"] = np.ascontiguousarray(arrs["x1"][b])
        in_maps.append(m)

    res = run_bass_kernel_spmd(nc, in_maps, core_ids=list(range(N_CORES)))
    out = np.stack([res.results[b]["out"] for b in range(N_CORES)], axis=0)
    return out.astype(np.float32)



# revision 2
# speedup vs baseline: 1.6164x; 1.6164x over previous
"""Trainium2 Bass kernel for nn_BlockDirectTwice (dual-branch transformer block).

Sharding: data-parallel over batch. 8 batch elements -> 8 NeuronCores; every
core runs the full block (two LN+MHA branches, blend, LN, MLP, residuals) on
its own [S, D] slab. No collectives.

Numerics v2: attention projections (q/k/v/o) and the probs@V contraction run
in fp8-e4m3 with DoubleRow (2 contraction rows per PE cell); scores q@k stay
bf16 (row-tiled 2 heads across the PE array); the MLP stays bf16; LayerNorm
statistics, softmax denominators and the residual stream stay fp32.

Scale bookkeeping (TRN fp8 overflows to inf at +-240, so ranges matter):
  weights dequant 1/S_W, v rows carry S_V, probs carry S_E (=exp scale,
  cancels in the softmax ratio), normalized ctx carries S_C via the
  reciprocal, and the wo accumulate divides by S_C*S_W.
"""

import os

import numpy as np
import ml_dtypes

B, S, D, H, DFF = 8, 1024, 768, 12, 3072
HD = D // H          # 64
P = 128
KD = D // P          # 6   K-subtiles over D
KD2 = KD // 2        # 3   fp8 DoubleRow K-pairs over D
KF = DFF // P        # 24  K-subtiles over DFF
NT = S // P          # 8   token tiles
NTP = NT // 2        # 4   token-tile pairs (DoubleRow ctx)
NPAIR = H // 2       # 6   head pairs
VW = 65              # v rows per head incl. ones row
VPAD = 784           # H*VW=780 padded so the t-stride is 16B-aligned
UP, MID = 0.6, 0.4
EPS = 1e-6
N_CORES = 8
ATT_SCALE = 1.0 / np.sqrt(HD)  # 0.125

S_W = 64.0           # host premultiplier on fp8 weights
S_V = 8.0            # scale carried by v rows (and the ones column)
S_E = 8.0            # scale on exp probs: pq = S_E * exp(score)
LOG_S_E = float(np.log(S_E))
S_C = 16.0           # scale on normalized ctx (via ones column = S_V/S_C)

_CACHE = {}


def _split_cols(n):
    """Split n output columns into <=512 chunks."""
    out, c = [], 0
    while c < n:
        w = min(512, n - c)
        out.append((c, w))
        c += w
    return out


def _build_nc(cfg):
    """Build the per-core Bass program. cfg is a frozenset of feature flags."""
    from contextlib import ExitStack

    import concourse.bass as bass
    import concourse.tile as tile
    from concourse import bacc, mybir

    F32 = mybir.dt.float32
    BF16 = mybir.dt.bfloat16
    F8 = mybir.dt.float8e4
    U8 = mybir.dt.uint8
    AF = mybir.ActivationFunctionType
    ALU = mybir.AluOpType
    DR = mybir.MatmulPerfMode.DoubleRow

    has = lambda f: f in cfg
    repeat = 1
    for f in cfg:
        if f.startswith("repeat="):
            repeat = int(f.split("=")[1])

    nc = bacc.Bacc("TRN2", target_bir_lowering=False, debug=False)

    # ---------------- DRAM I/O ----------------
    x_dram = [
        nc.dram_tensor("x0", (S, D), F32, kind="ExternalInput"),
        nc.dram_tensor("x1", (S, D), F32, kind="ExternalInput"),
    ]
    w_dram = {}
    for br in (0, 1):
        for nm in ("wq", "wk", "wv", "wo"):
            w_dram[(br, nm)] = nc.dram_tensor(f"a{br}_{nm}", (D, D), F8,
                                              kind="ExternalInput")
    fc1_dram = nc.dram_tensor("fc1_w", (D, DFF), BF16, kind="ExternalInput")
    fc2_dram = nc.dram_tensor("fc2_w", (DFF, D), BF16, kind="ExternalInput")

    # optional non-trivial params (most are zeros/ones in this problem)
    opt_dram = {}
    for name, shape in [
        ("ln0_g", (D,)), ("ln0_b", (D,)), ("ln1_g", (D,)), ("ln1_b", (D,)),
        ("lnf_g", (D,)), ("lnf_b", (D,)),
        ("fc1_b", (DFF,)), ("fc2_b", (D,)),
        ("a0_bq", (D,)), ("a0_bk", (D,)), ("a0_bv", (D,)), ("a0_bo", (D,)),
        ("a1_bq", (D,)), ("a1_bk", (D,)), ("a1_bv", (D,)), ("a1_bo", (D,)),
    ]:
        if has(name):
            opt_dram[name] = nc.dram_tensor(name, shape, F32, kind="ExternalInput")

    out_dram = nc.dram_tensor("out", (S, D), F32, kind="ExternalOutput")

    def bcast_rows(src_ap, nparts):
        """DRAM row [1, n] (or [n]) -> AP broadcast over nparts partitions."""
        ap = list(src_ap.ap)
        if len(src_ap.shape) == 1:
            ap = [[0, nparts]] + ap
        else:
            ap = [[0, nparts]] + ap[1:]
        return bass.AP(tensor=src_ap.tensor, offset=src_ap.offset, ap=ap)

    with ExitStack() as ctx:
        tc = ctx.enter_context(tile.TileContext(nc))

        sb = ctx.enter_context(tc.tile_pool(name="sb", bufs=1))
        wpool = ctx.enter_context(tc.tile_pool(name="w", bufs=3))
        lnp = ctx.enter_context(tc.tile_pool(name="ln", bufs=2))
        qkp = ctx.enter_context(tc.tile_pool(name="qk", bufs=3))
        xtp = ctx.enter_context(tc.tile_pool(name="xt", bufs=1))
        prp = ctx.enter_context(tc.tile_pool(name="probs", bufs=2))
        outp = ctx.enter_context(tc.tile_pool(name="out", bufs=2))
        psmm = ctx.enter_context(tc.tile_pool(name="psmm", bufs=4, space="PSUM"))
        pssc = ctx.enter_context(tc.tile_pool(name="pssc", bufs=2, space="PSUM"))
        dram = ctx.enter_context(tc.tile_pool(name="dram", bufs=1, space="DRAM"))

        loop_cm = tc.For_i(0, repeat, 1) if repeat > 1 else None
        if loop_cm is not None:
            ctx.enter_context(loop_cm)

        expb = sb.tile([P, 1], F32, tag="expb")
        nc.vector.memset(expb, LOG_S_E)

        # persistent big tensors
        h_tm = sb.tile([P, NT, D], F32, tag="h_tm")
        ctx8 = sb.tile([P, KD, S], F8, tag="ctx8")       # normalized ctx, *S_C
        v_aug = sb.tile([P, NT, VPAD], F8, tag="v")
        fc1_sb = sb.tile([P, KD, DFF], BF16, tag="fc1")
        fc2_sb = sb.tile([P, KF, D], BF16, tag="fc2")

        # optional broadcast tiles for per-feature (free-dim) params
        bcast_sb = {}
        for name in ("ln0_g", "ln0_b", "ln1_g", "ln1_b", "lnf_g", "lnf_b",
                     "a0_bv", "a1_bv", "a0_bo", "a1_bo", "fc2_b"):
            if has(name):
                t = sb.tile([P, D], F32, tag=f"bc_{name}")
                nc.gpsimd.dma_start(t, bcast_rows(opt_dram[name].ap(), P))
                bcast_sb[name] = t
        for name in ("a0_bv", "a1_bv"):
            if name in bcast_sb:  # v rows carry S_V
                nc.vector.tensor_scalar_mul(bcast_sb[name], bcast_sb[name], S_V)
        # per-partition bias tiles (feature-major layouts)
        pp_sb = {}
        for name, kk in (("a0_bq", KD), ("a0_bk", KD), ("a1_bq", KD),
                         ("a1_bk", KD), ("fc1_b", KF)):
            if has(name):
                t = sb.tile([P, kk], F32, tag=f"pp_{name}")
                nc.sync.dma_start(t, opt_dram[name].ap().rearrange("(m p) -> p m", p=P))
                pp_sb[name] = t

        # identity for PE-mode transposes
        from concourse.masks import make_identity
        ident = sb.tile([P, P], BF16, tag="ident")
        make_identity(nc, ident)

        def emit_ln(x_f32, xT_dest, t, gname, bname):
            """LayerNorm x_f32 [P, D] (in-place scratch) -> bf16, transposed on
            the PE into xT_dest[:, j, t*128:(t+1)*128] (any dtype: the PSUM
            drain copy converts)."""
            stats = lnp.tile([P, 3, 6], F32, tag="stats")
            for sg in range(3):
                nc.vector.bn_stats(stats[:, sg, :], x_f32[:, sg * 256:(sg + 1) * 256])
            mv = lnp.tile([P, 2], F32, tag="mv")
            nc.vector.bn_aggr(mv, stats)
            # rstd = 1/sqrt(var+eps), DVE-only (quake init + 2 Newton steps):
            # keeps the ACT table set untouched (exp stays resident).
            rstd = lnp.tile([P, 1], F32, tag="rstd")
            vh = lnp.tile([P, 1], F32, tag="rs_vh")
            nc.vector.tensor_scalar(vh, mv[:, 1:2], EPS, 0.5, ALU.add, ALU.mult)
            yi = lnp.tile([P, 1], mybir.dt.int32, tag="rs_yi")
            # quake seed from the bits of u = var (Newton uses h = u/2)
            nc.vector.tensor_scalar(yi, mv[:, 1:2].bitcast(mybir.dt.int32), 1, None,
                                    ALU.logical_shift_right)
            y0 = lnp.tile([P, 1], F32, tag="rs_y0")
            nc.vector.tensor_scalar(yi, yi, -1, None, ALU.bitwise_xor)
            nc.vector.tensor_scalar(y0.bitcast(mybir.dt.int32), yi, 0x5f3759e0, None,
                                    ALU.add)
            t1 = lnp.tile([P, 1], F32, tag="rs_t1")
            for _ in range(2):
                nc.vector.tensor_tensor(t1, y0, y0, ALU.mult)
                nc.vector.tensor_tensor(t1, t1, vh, ALU.mult)
                nc.vector.tensor_scalar(t1, t1, -1.0, 1.5, ALU.mult, ALU.add)
                nc.vector.tensor_tensor(y0, y0, t1, ALU.mult)
            nc.vector.tensor_copy(rstd, y0)
            nc.vector.tensor_scalar(x_f32, x_f32, mv[:, 0:1], None, ALU.subtract)
            xln = lnp.tile([P, D], BF16, tag="xln")
            if has(gname):
                nc.vector.tensor_scalar_mul(x_f32, x_f32, rstd[:])
                nc.vector.tensor_tensor(x_f32, x_f32, bcast_sb[gname], ALU.mult)
                if has(bname):
                    nc.vector.tensor_tensor(xln, x_f32, bcast_sb[bname], ALU.add)
                else:
                    nc.vector.tensor_copy(xln, x_f32)
            elif has(bname):
                nc.vector.tensor_scalar_mul(x_f32, x_f32, rstd[:])
                nc.vector.tensor_tensor(xln, x_f32, bcast_sb[bname], ALU.add)
            else:
                nc.vector.tensor_scalar_mul(xln, x_f32, rstd[:])
            emit_tr(xln, xT_dest, t)

        def emit_tr(xln, xT_dest, t):
            """6 PE transposes into one PSUM bank, one batched DVE drain."""
            pst = psmm.tile([P, 512], F32, tag="mm")
            pb = pst.bitcast(BF16)
            for j in range(KD):
                nc.tensor.transpose(pb[:, j * P:(j + 1) * P],
                                    xln[:, j * P:(j + 1) * P], ident)
            nc.vector.tensor_copy(
                xT_dest[:, :, t * P:(t + 1) * P],
                pb[:, 0:KD * P].rearrange("p (k c) -> p k c", c=P))

        def ln_feed(mvall, t, src):
            stats = lnp.tile([P, 3, 6], F32, tag="stats")
            for sg in range(3):
                nc.vector.bn_stats(stats[:, sg, :],
                                   src[:, sg * 256:(sg + 1) * 256])
            nc.vector.bn_aggr(mvall[:, t, :], stats)

        def ln_stats_batched(src_of_t):
            """bn stats per tile + one batched quake/Newton rstd over [P, NT].
            Returns (rstd_all, negmurstd) [P, NT] f32."""
            mvall = lnp.tile([P, NT, 2], F32, tag="mvall")
            for t in range(NT):
                ln_feed(mvall, t, src_of_t(t))
            return ln_newton(mvall)

        def ln_newton(mvall):
            I32 = mybir.dt.int32
            var = mvall[:, :, 1:2]
            mu = mvall[:, :, 0:1]
            vh = lnp.tile([P, NT], F32, tag="vhall")
            nc.vector.tensor_scalar(vh, var, EPS, 0.5, ALU.add, ALU.mult)
            yi = lnp.tile([P, NT], I32, tag="yiall")
            nc.vector.tensor_scalar(yi, var.bitcast(I32), 1, None,
                                    ALU.logical_shift_right)
            y0 = lnp.tile([P, NT], F32, tag="y0all")
            nc.vector.tensor_scalar(yi, yi, -1, None, ALU.bitwise_xor)
            nc.vector.tensor_scalar(y0.bitcast(I32), yi, 0x5f3759e0, None,
                                    ALU.add)
            t1 = lnp.tile([P, NT], F32, tag="t1all")
            for _ in range(2):
                nc.vector.tensor_tensor(t1, y0, y0, ALU.mult)
                nc.vector.tensor_tensor(t1, t1, vh, ALU.mult)
                nc.vector.tensor_scalar(t1, t1, -1.0, 1.5, ALU.mult, ALU.add)
                nc.vector.tensor_tensor(y0, y0, t1, ALU.mult)
            nmr = lnp.tile([P, NT], F32, tag="nmrall")
            nc.vector.scalar_tensor_tensor(nmr, mu, -1.0, y0, ALU.mult, ALU.mult)
            return y0, nmr

        def ln_apply(src, xT_dest, t, rstd, nmr):
            """xln = src*rstd - mu*rstd on ACT (Identity w/ AP scale+bias)."""
            xln = lnp.tile([P, D], BF16, tag="xln")
            nc.scalar.activation(xln, src, AF.Identity,
                                 bias=nmr[:, t:t + 1], scale=rstd[:, t:t + 1])
            emit_tr(xln, xT_dest, t)

        def stage_A1_phase1():
            """Branch-1 loads + blend + LN stats, hoisted before branch-0's
            attention so the DVE work hides under the ACT-bound window.
            Returns a closure that finishes the LN into xT1."""
            xb = sb.tile([P, NT, D], BF16, tag="xb")
            mvall = lnp.tile([P, NT, 2], F32, tag="mvall")
            for t in range(NT):
                xt = lnp.tile([P, D], F32, tag="x_tm")
                nc.sync.dma_start(xt, x_dram[1].ap()[t * P:(t + 1) * P, :])
                ln_feed(mvall, t, xt)
                nc.vector.tensor_copy(xb[:, t, :], xt)
                nc.vector.scalar_tensor_tensor(h_tm[:, t, :], xt, MID,
                                               h_tm[:, t, :], ALU.mult, ALU.add)

            def phase2(xT_dest):
                rstd, nmr = ln_newton(mvall)
                for t in range(NT):
                    ln_apply(xb[:, t, :], xT_dest, t, rstd, nmr)
            return phase2

        def stage_A(br, xT_dest):
            """Load x_br, LN, transpose; accumulate blend into h_tm."""
            g, b = (f"ln{br}_g", f"ln{br}_b")
            fast = not (has(g) or has(b) or has("lnold"))
            if fast and br == 0:
                # LN is scale-invariant: run it on h_tm = UP*x0 directly
                for t in range(NT):
                    xt = lnp.tile([P, D], F32, tag="x_tm")
                    nc.sync.dma_start(xt, x_dram[br].ap()[t * P:(t + 1) * P, :])
                    nc.vector.tensor_scalar_mul(h_tm[:, t, :], xt, UP)
                rstd, nmr = ln_stats_batched(lambda t: h_tm[:, t, :])
                for t in range(NT):
                    ln_apply(h_tm[:, t, :], xT_dest, t, rstd, nmr)
                return
            if fast:
                xb = sb.tile([P, NT, D], BF16, tag="xb")
                for t in range(NT):
                    xt = lnp.tile([P, D], F32, tag="x_tm")
                    nc.sync.dma_start(xt, x_dram[br].ap()[t * P:(t + 1) * P, :])
                    nc.vector.tensor_copy(xb[:, t, :], xt)
                    nc.vector.scalar_tensor_tensor(h_tm[:, t, :], xt, MID,
                                                   h_tm[:, t, :], ALU.mult, ALU.add)
                rstd, nmr = ln_stats_batched(lambda t: xb[:, t, :])
                for t in range(NT):
                    ln_apply(xb[:, t, :], xT_dest, t, rstd, nmr)
                return
            for t in range(NT):
                xt = lnp.tile([P, D], F32, tag="x_tm")
                nc.sync.dma_start(xt, x_dram[br].ap()[t * P:(t + 1) * P, :])
                if br == 0:
                    nc.vector.tensor_scalar_mul(h_tm[:, t, :], xt, UP)
                else:
                    nc.vector.scalar_tensor_tensor(h_tm[:, t, :], xt, MID,
                                                   h_tm[:, t, :], ALU.mult, ALU.add)
                emit_ln(xt, xT_dest, t, g, b)

        def load_w(br, nm):
            t = wpool.tile([P, KD, D], F8, tag="w768")
            nc.gpsimd.dma_start(t, w_dram[(br, nm)].ap().rearrange("(ko p) n -> p ko n", p=P))
            return t

        def mm_dr(ps, lhsT, rhs_of_kk, cw):
            """fp8 DoubleRow accumulation over the 3 K-pairs of D."""
            for kk in range(KD2):
                nc.tensor.matmul(ps[:, :cw], lhsT=lhsT(kk), rhs=rhs_of_kk(kk),
                                 start=(kk == 0), stop=(kk == KD2 - 1),
                                 perf_mode=DR)

        def stage_BC(br, xT, pending_wo=None):
            """V/Q/K projections + attention, with next-pair projection chunks
            interleaved into the attention loop so the in-order PE queue
            stays dense while ACT computes exps. Returns a closure that emits
            the wo projection (deferred into the next branch's warmup)."""
            wv = load_w(br, "wv")
            wq = load_w(br, "wq")
            v_view = v_aug[:, :, 0:H * VW].rearrange("p t (h c) -> p t h c", c=VW)
            nc.vector.memset(v_view[:, :, :, HD:VW], S_V / S_C)
            for t in range(NT):
                for c0, cw in _split_cols(D):
                    ps = psmm.tile([P, 512], F32, tag="mm")
                    mm_dr(ps, lambda kk: xT[:, 2 * kk:2 * kk + 2, t * P:(t + 1) * P],
                          lambda kk: wv[:, 2 * kk:2 * kk + 2, c0:c0 + cw], cw)
                    nh = cw // HD
                    h0 = c0 // HD
                    src = ps[:, :cw].rearrange("p (h c) -> p h c", c=HD)
                    dst = v_view[:, t, h0:h0 + nh, 0:HD]
                    bias_key = f"a{br}_bv"
                    if bias_key in bcast_sb:
                        bcv = bcast_sb[bias_key][:, c0:c0 + cw].rearrange(
                            "p (h c) -> p h c", c=HD)
                        nc.vector.scalar_tensor_tensor(dst, src, S_V / S_W, bcv,
                                                       ALU.mult, ALU.add)
                    else:
                        nc.vector.tensor_scalar_mul(dst, src, S_V / S_W)
            extra_q = []
            if pending_wo is not None:
                extra_q = pending_wo()
            wk = load_w(br, "wk")
            if has("noattn"):
                nc.vector.memset(ctx8, 0.25)

            def proj_chunks(pr, qp, kp):
                """12 closures, each one (which, col-chunk) psum of pair pr."""
                chunks = []
                for (which, wt, dest) in (("q", wq, qp), ("k", wk, kp)):
                    for c0, cw in _split_cols(S):
                        def emit(which=which, wt=wt, dest=dest, c0=c0, cw=cw,
                                 pr=pr):
                            ps = psmm.tile([P, 512], F32, tag="mm")
                            mm_dr(ps,
                                  lambda kk: wt[:, 2 * kk:2 * kk + 2, pr * P:(pr + 1) * P],
                                  lambda kk: xT[:, 2 * kk:2 * kk + 2, c0:c0 + cw], cw)
                            nc.vector.tensor_scalar_mul(
                                dest[:, c0:c0 + cw], ps[:, :cw], 1.0 / S_W)
                            bias_key = f"a{br}_b{which}"
                            if bias_key in pp_sb and c0 + cw >= S:
                                nc.vector.tensor_scalar_add(
                                    dest, dest, pp_sb[bias_key][:, pr:pr + 1])
                        chunks.append(emit)
                return chunks

            if not has("noattn"):
                # prime pair 0 (nothing to interleave with yet)
                cur_qp = qkp.tile([P, S], F8, tag="qpair", bufs=2, name="qp0")
                cur_kp = qkp.tile([P, S], F8, tag="kpair", bufs=2, name="kp0")
                for ch in proj_chunks(0, cur_qp, cur_kp):
                    ch()
                for pr in range(NPAIR):
                    qp, kp = cur_qp, cur_kp
                    fillers = []
                    if pr + 1 < NPAIR:
                        cur_qp = qkp.tile([P, S], F8, tag="qpair", bufs=2,
                                          name=f"qp{pr + 1}")
                        cur_kp = qkp.tile([P, S], F8, tag="kpair", bufs=2,
                                          name=f"kp{pr + 1}")
                        fillers = proj_chunks(pr + 1, cur_qp, cur_kp)
                    nfill = 0
                    cb = outp.tile([P, S], BF16, tag="ctxb", bufs=1)
                    den_dram = dram.tile([2, 2, 512], BF16, name=f"den{br}_{pr}")
                    for n in range(2):
                        n0 = n * 512
                        ps_c = [psmm.tile([P, 512], F32, tag="mm", name=f"ps_c{hh}")
                                for hh in range(2)]

                        def ctx_step(tau, pq2):
                            for hh in range(2):
                                h = 2 * pr + hh
                                nc.tensor.matmul(
                                    ps_c[hh][0:VW, :],
                                    lhsT=v_aug[:, 2 * tau:2 * tau + 2,
                                               h * VW:(h + 1) * VW],
                                    rhs=pq2[:, :, hh, :],
                                    start=(tau == 0), stop=(tau == NTP - 1),
                                    perf_mode=DR)

                        LAG = 1
                        pending = []
                        for tau in range(NTP):
                            pq2 = prp.tile([P, 2, 2, 512], F8, tag="probs")
                            for i in range(2):
                                t = 2 * tau + i
                                ps_s = pssc.tile([P, 2, 512], F32, tag="sc")
                                for hh in range(2):
                                    b0 = hh * HD
                                    nc.tensor.matmul(
                                        ps_s[:, hh, :],
                                        lhsT=kp[b0:b0 + HD, t * P:(t + 1) * P],
                                        rhs=qp[b0:b0 + HD, n0:n0 + 512],
                                        start=True, stop=True)
                                if has("dveexp"):
                                    # head 0 on ACT, head 1 on DVE via the
                                    # Schraudolph fp8-bit trick
                                    nc.scalar.activation(pq2[:, i, 0, :],
                                                         ps_s[:, 0, :], AF.Exp,
                                                         bias=expb,
                                                         scale=float(ATT_SCALE))
                                    u8 = pq2.bitcast(U8)
                                    nc.vector.tensor_scalar(
                                        u8[:, i, 1, :], ps_s[:, 1, :],
                                        11.5415603 * ATT_SCALE, 79.651,
                                        ALU.mult, ALU.add)
                                elif has("dveexp2") and n == 1:
                                    # whole block on DVE: no mixed-engine
                                    # writes into one pq2 tile
                                    u8 = pq2.bitcast(U8)
                                    nc.vector.tensor_scalar(
                                        u8[:, i, :, :], ps_s,
                                        11.5415603 * ATT_SCALE, 79.651,
                                        ALU.mult, ALU.add)
                                else:
                                    nc.scalar.activation(pq2[:, i, :, :], ps_s,
                                                         AF.Exp, bias=expb,
                                                         scale=float(ATT_SCALE))
                            pending.append((tau, pq2))
                            if len(pending) > LAG:
                                ctx_step(*pending.pop(0))
                            # keep PE dense: next-pair projection chunks spread
                            # across the attention loop
                            want = ((n * NTP + tau + 1) * len(fillers)) // (2 * NTP)
                            while nfill < want:
                                fillers[nfill]()
                                nfill += 1
                            # extra work (prev-branch wo) must fully drain
                            # before this branch's first ctx8 write: 2/slot
                            # finishes within pair 0
                            for _ in range(min(2, len(extra_q))):
                                extra_q.pop(0)()
                        for item in pending:
                            ctx_step(*item)
                        dstage = lnp.tile([VW, 2, 512], BF16, tag="dstage")
                        for hh in range(2):
                            nc.vector.tensor_copy(
                                cb[hh * HD:(hh + 1) * HD, n0:n0 + 512],
                                ps_c[hh][0:HD, :])
                            nc.vector.tensor_copy(dstage[HD:VW, hh, :],
                                                  ps_c[hh][HD:VW, :])
                        nc.scalar.dma_start(den_dram[:, n, :],
                                            dstage[HD:VW, :, :])
                    while nfill < len(fillers):
                        fillers[nfill]()
                        nfill += 1
                    while extra_q:  # guard: ctx8 is overwritten below
                        extra_q.pop(0)()
                    # normalize this pair now (pipelines with the next pair):
                    # broadcast denominators 1->64 partitions via a DRAM
                    # bounce, then ctx8 = cb * (1/den); ones col carries
                    # S_V/S_C so ctx8 = S_C * ctx_true
                    rb = outp.tile([P, S], F32, tag="recipB", bufs=1)
                    for hh in range(2):
                        nc.gpsimd.dma_start(
                            rb[hh * HD:(hh + 1) * HD, :],
                            bcast_rows(den_dram[hh:hh + 1, :, :].rearrange(
                                "h a b -> h (a b)"), HD))
                    nc.vector.reciprocal_approx_fast(rb, rb)
                    nc.vector.tensor_tensor(ctx8[:, pr, :], cb, rb, ALU.mult)

            for ch in extra_q:
                ch()

            def emit_wo():
                wo = load_w(br, "wo")
                scale = UP if br == 0 else MID
                return _wo_chunks(br, wo, scale)

            return emit_wo

        def _wo_chunks(br, wo, scale):
            bo_key = f"a{br}_bo"
            chunks = []
            for t in range(NT):
                for c0, cw in _split_cols(D):
                    chunks.append(lambda t=t, c0=c0, cw=cw: _wo_one(
                        br, wo, scale, bo_key, t, c0, cw))
            return chunks

        def _wo_one(br, wo, scale, bo_key, t, c0, cw):
                    ps = psmm.tile([P, 512], F32, tag="mm")
                    mm_dr(ps, lambda kk: ctx8[:, 2 * kk:2 * kk + 2, t * P:(t + 1) * P],
                          lambda kk: wo[:, 2 * kk:2 * kk + 2, c0:c0 + cw], cw)
                    if bo_key in bcast_sb:
                        tmp = lnp.tile([P, D], F32, tag="wo_tmp")
                        nc.vector.scalar_tensor_tensor(
                            tmp[:, :cw], ps[:, :cw], 1.0 / (S_C * S_W),
                            bcast_sb[bo_key][:, c0:c0 + cw], ALU.mult, ALU.add)
                        nc.vector.scalar_tensor_tensor(
                            h_tm[:, t, c0:c0 + cw], tmp[:, :cw], float(scale),
                            h_tm[:, t, c0:c0 + cw], ALU.mult, ALU.add)
                    else:
                        nc.vector.scalar_tensor_tensor(
                            h_tm[:, t, c0:c0 + cw], ps[:, :cw],
                            float(scale) / (S_C * S_W),
                            h_tm[:, t, c0:c0 + cw], ALU.mult, ALU.add)

        # ---------------- emit program ----------------
        fastA1 = not (has("ln1_g") or has("ln1_b") or has("lnold"))
        xT0 = xtp.tile([P, KD, S], F8, tag="xT")
        stage_A(0, xT0)
        ph2 = stage_A1_phase1() if fastA1 else None
        wo0 = stage_BC(0, xT0)
        # fc weights load mid-kernel: after branch-0's attention weights,
        # well before the MLP needs them
        nc.gpsimd.dma_start(fc1_sb, fc1_dram.ap().rearrange("(ko p) n -> p ko n", p=P))
        nc.gpsimd.dma_start(fc2_sb, fc2_dram.ap().rearrange("(ko p) n -> p ko n", p=P))
        xT1 = xtp.tile([P, KD, S], F8, tag="xT")
        if fastA1:
            ph2(xT1)
        else:
            stage_A(1, xT1)
        wo1 = stage_BC(1, xT1, pending_wo=wo0)
        for ch in wo1():
            ch()

        # LNf -> hT (bf16: the MLP stays bf16)
        hT = xtp.tile([P, KD, S], BF16, tag="hT")
        if not (has("lnf_g") or has("lnf_b") or has("lnold")):
            rstd, nmr = ln_stats_batched(lambda t: h_tm[:, t, :])
            for t in range(NT):
                ln_apply(h_tm[:, t, :], hT, t, rstd, nmr)
        else:
            for t in range(NT):
                hc = lnp.tile([P, D], F32, tag="x_tm")
                nc.vector.tensor_copy(hc, h_tm[:, t, :])
                emit_ln(hc, hT, t, "lnf_g", "lnf_b")

        # MLP: fc1+gelu then fc2+residual, in token chunks of 256
        if has("nomlp"):
            for t in range(NT):
                o_t = outp.tile([P, D], F32, tag="out_t", bufs=1)
                nc.vector.tensor_copy(o_t, h_tm[:, t, :])
                nc.gpsimd.dma_start(out_dram.ap()[t * P:(t + 1) * P, :], o_t)
        for nn in range(2 if not has("nomlp") else 0):
            c0 = nn * 512
            gT = xtp.tile([P, KF, 512], BF16, tag="gT")
            for m in range(KF):
                ps = psmm.tile([P, 512], F32, tag="mm")
                for k in range(KD):
                    nc.tensor.matmul(ps, lhsT=fc1_sb[:, k, m * P:(m + 1) * P],
                                     rhs=hT[:, k, c0:c0 + 512],
                                     start=(k == 0), stop=(k == KD - 1))
                bias = pp_sb["fc1_b"][:, m:m + 1] if "fc1_b" in pp_sb else 0.0
                nc.scalar.activation(gT[:, m, :], ps, AF.Gelu, bias=bias)
            for tl in range(4):
                t = 4 * nn + tl
                o_t = outp.tile([P, D], F32, tag="out_t", bufs=1)
                for oc0, ocw in _split_cols(D):
                    ps = psmm.tile([P, 512], F32, tag="mm")
                    for k in range(KF):
                        nc.tensor.matmul(
                            ps[:, :ocw], lhsT=gT[:, k, tl * P:(tl + 1) * P],
                            rhs=fc2_sb[:, k, oc0:oc0 + ocw],
                            start=(k == 0), stop=(k == KF - 1))
                    if "fc2_b" in bcast_sb:
                        nc.vector.tensor_tensor(ps[:, :ocw], ps[:, :ocw],
                                                bcast_sb["fc2_b"][:, oc0:oc0 + ocw],
                                                ALU.add)
                    nc.vector.tensor_tensor(o_t[:, oc0:oc0 + ocw], ps[:, :ocw],
                                            h_tm[:, t, oc0:oc0 + ocw], ALU.add)
                nc.gpsimd.dma_start(out_dram.ap()[t * P:(t + 1) * P, :], o_t)

    nc.compile()
    return nc


def _prep_inputs(inputs):
    """Host-side prep: detect trivial params, cast weights to fp8/bf16."""
    bf16 = ml_dtypes.bfloat16
    f8 = ml_dtypes.float8_e4m3
    cfg = set()
    arrs = {}
    for name in ("x0", "x1"):
        arrs[name] = np.ascontiguousarray(np.asarray(inputs[name], dtype=np.float32))
    for br in (0, 1):
        for nm in ("wq", "wk", "wv", "wo"):
            key = f"a{br}_{nm}"
            w = np.asarray(inputs[key], dtype=np.float32) * S_W
            arrs[key] = np.ascontiguousarray(
                np.clip(w, -240.0, 240.0).astype(f8))
    arrs["fc1_w"] = np.ascontiguousarray(
        np.asarray(inputs["fc1_w"], dtype=np.float32).astype(bf16))
    arrs["fc2_w"] = np.ascontiguousarray(
        np.asarray(inputs["fc2_w"], dtype=np.float32).astype(bf16))
    for name, trivial in [
        ("ln0_g", 1.0), ("ln0_b", 0.0), ("ln1_g", 1.0), ("ln1_b", 0.0),
        ("lnf_g", 1.0), ("lnf_b", 0.0), ("fc1_b", 0.0), ("fc2_b", 0.0),
        ("a0_bq", 0.0), ("a0_bk", 0.0), ("a0_bv", 0.0), ("a0_bo", 0.0),
        ("a1_bq", 0.0), ("a1_bk", 0.0), ("a1_bv", 0.0), ("a1_bo", 0.0),
    ]:
        a = np.asarray(inputs[name], dtype=np.float32)
        if not np.all(a == trivial):
            cfg.add(name)
            arrs[name] = np.ascontiguousarray(a)
    return cfg, arrs


def kernel(**inputs):
    from concourse.bass_utils import run_bass_kernel_spmd

    cfg, arrs = _prep_inputs(inputs)
    cfg |= {f for f in os.environ.get("K2_FLAGS", "").split(",") if f}
    key = frozenset(cfg)
    if key not in _CACHE:
        _CACHE[key] = _build_nc(key)
    nc = _CACHE[key]

    shared = {k: v for k, v in arrs.items() if k not in ("x0", "x1")}
    in_maps = []
    for b in range(N_CORES):
        m = dict(shared)
        m["x0"] = np.ascontiguousarray(arrs["x0"][b])
        m["x1"] = np.ascontiguousarray(arrs["x1"][b])
        in_maps.append(m)

    res = run_bass_kernel_spmd(nc, in_maps, core_ids=list(range(N_CORES)))
    out = np.stack([res.results[b]["out"] for b in range(N_CORES)], axis=0)
    return out.astype(np.float32)
